# revision 13
# baseline (speedup 1.0000x reference)
"""Trainium2 Bass kernel for nn_Decoder (gnn_message_passing).

Math (per batch b, agent a):
    s[b,a]  = abs_actions[b, idx[b,a]]                     (gather, idx < 16)
    z[b,a,:] = s[b,a] * W1[0,:] + embed[a,:] @ W1[1:,:] + b1
    out[b,a,:] = relu(z) @ W2 + b2

Device algorithm (per core, hT layout z[h, a], pure data-parallel over B):
  - e[h,a] = (embed @ W1[1:]).T + b1 is batch-independent; it is computed
    once on device (matmuls from W1h / embT) into 3 PSUM tiles that stay
    RESIDENT for the whole kernel.
  - The gather is folded into a rank-64 matmul: the per-batch term is
    v_b = U_b.T @ onehot_b with U_b = outer(abs_row_b, W1[0]); the host ships
    the one-hot encodings (bf16 exact) and U split into bf16 hi/lo halves so
    the bf16 matmul reproduces the f32 product almost exactly.  Each batch
    issues a single "transition" matmul pair (2 h-chunks, K=64) whose
    stacked rhs holds [-onehot_{b-3}; +onehot_b] twice: it simultaneously
    removes the previous occupant's contribution from the rotating PSUM tile
    and adds the new batch's, so PSUM always holds z = e + v_b right after.
    fp32 PSUM makes the add/remove round-trip drift negligible (~1e-7).
  - relu evacuation PSUM->SBUF alternates between ScalarE (ACTIVATE Relu)
    and VectorE (tensor_scalar max 0), the two engines that can read PSUM.
  - Stage 2 (h @ W2, OUT=2) packs 64 batches into one PSUM bank: W2 sits in
    zero-padded 32-column "slot" tiles targeting column-strip j via
    tile_position=(0,32j); batch gg lands on partitions 32j+2s+{0,1}.
    One copy + DMA evacuates 64 batches of output at once; the host
    unpermutes the [blocks,128,512] scratch layout.
"""

import numpy as np
import ml_dtypes

import concourse.bass as bass
import concourse.bacc as bacc
import concourse.mybir as mybir
import concourse.tile as tile
from concourse import bass_utils

F32 = mybir.dt.float32
BF16 = mybir.dt.bfloat16

B, A, NABS, E, H, OUT = 2048, 512, 16, 256, 256, 2
NCORES = 8
BC = B // NCORES  # batches per core
NE = 3  # rotating resident-e PSUM tiles

AF = mybir.ActivationFunctionType
ALU = mybir.AluOpType


def _build(nb: int):
    """Build the per-core module processing nb batches."""
    assert nb % 4 == 0
    block = min(64, nb)  # batches accumulated per stage-2 psum bank
    nc = bacc.Bacc(
        "TRN2", target_bir_lowering=False, debug=False, num_devices=NCORES
    )

    ohpm_d = nc.dram_tensor("ohpm", [nb, 64, 512], BF16, kind="ExternalInput").ap()
    u64_d = nc.dram_tensor("u64", [nb, 64, H], BF16, kind="ExternalInput").ap()
    w1h_d = nc.dram_tensor("w1h", [E, H], F32, kind="ExternalInput").ap()
    b1_d = nc.dram_tensor("b1r", [1, H], F32, kind="ExternalInput").ap()
    embT_d = nc.dram_tensor("embT", [E, A], F32, kind="ExternalInput").ap()
    w2sl_d = nc.dram_tensor("w2sl", [2, 128, 512], BF16, kind="ExternalInput").ap()
    b2c_d = nc.dram_tensor("b2c", [128, 1], F32, kind="ExternalInput").ap()
    out_d = nc.dram_tensor(
        "out", [nb // block, 128, 512], F32, kind="ExternalOutput"
    ).ap()

    with tile.TileContext(nc) as tc:
        with (
            tc.tile_pool(name="const", bufs=1) as cpool,
            tc.tile_pool(name="ohb", bufs=6) as ohpool,
            tc.tile_pool(name="u", bufs=4) as upool,
            tc.tile_pool(name="h", bufs=6) as hpool,
            tc.tile_pool(name="osb", bufs=2) as opool,
            tc.tile_pool(name="epool", bufs=NE, space="PSUM") as epool,
            tc.tile_pool(name="o2", bufs=2, space="PSUM") as o2pool,
        ):
            # ---- resident constants ----
            w1h0 = cpool.tile([128, H], F32, tag="w1h0")
            nc.sync.dma_start(w1h0[:], w1h_d[0:128, :])
            w1h1 = cpool.tile([128, H], F32, tag="w1h1")
            nc.sync.dma_start(w1h1[:], w1h_d[128:256, :])
            b1sb = cpool.tile([1, H], F32, tag="b1sb")
            nc.sync.dma_start(b1sb[:], b1_d[:])
            embT0 = cpool.tile([128, A], F32, tag="embT0")
            nc.sync.dma_start(embT0[:], embT_d[0:128, :])
            embT1 = cpool.tile([128, A], F32, tag="embT1")
            nc.sync.dma_start(embT1[:], embT_d[128:256, :])
            w2sl0 = cpool.tile([128, 512], BF16, tag="w2sl0")
            nc.sync.dma_start(w2sl0[:], w2sl_d[0])
            w2sl1 = cpool.tile([128, 512], BF16, tag="w2sl1")
            nc.sync.dma_start(w2sl1[:], w2sl_d[1])
            b2c = cpool.tile([128, 1], F32, tag="b2c")
            nc.sync.dma_start(b2c[:], b2c_d[:])
            ones = cpool.tile([1, A], F32, tag="ones")
            nc.vector.memset(ones[:], 1.0)

            w1h = [w1h0, w1h1]
            embT = [embT0, embT1]
            w2sl = [w2sl0, w2sl1]

            # ---- seed the resident e tiles:  e[h, a] = W1h.T @ embT + b1 ----
            etiles = [
                epool.tile([128, 1024], F32, tag="e", name=f"etile{t}")
                for t in range(NE)
            ]
            for et in etiles:
                for c in range(2):  # h-chunk
                    for c2 in range(2):  # contraction (embedding dim) chunk
                        nc.tensor.matmul(
                            et[:, c * 512 : (c + 1) * 512],
                            w1h[c2][:, c * 128 : (c + 1) * 128],
                            embT[c2][:],
                            start=(c2 == 0),
                            stop=False,
                            skip_group_check=True,
                        )
                    nc.tensor.matmul(
                        et[:, c * 512 : (c + 1) * 512],
                        b1sb[:, c * 128 : (c + 1) * 128],
                        ones[:],
                        start=False,
                        stop=True,
                        skip_group_check=True,
                    )

            # ---- batch loop ----
            o2 = None
            ohb4 = None
            hts = [None] * 4
            for b in range(nb):
                if b % 4 == 0:
                    ohb4 = ohpool.tile([64, 2048], BF16, tag="ohb")
                    nc.sync.dma_start(
                        ohb4[:].rearrange("p (t c) -> p t c", t=4),
                        ohpm_d[b : b + 4].rearrange("t p c -> p t c"),
                    )
                    u4 = upool.tile([64, 4 * H], BF16, tag="U")
                    nc.sync.dma_start(
                        u4[:].rearrange("p (t c) -> p t c", t=4),
                        u64_d[b : b + 4].rearrange("t p c -> p t c"),
                    )
                ohb = ohb4[:, (b % 4) * 512 : (b % 4) * 512 + 512]
                U = u4[:, (b % 4) * H : (b % 4) * H + H]

                et = etiles[b % NE]
                # transition: E += -U_{b-NE}.T @ oh_{b-NE} + U_b.T @ oh_b
                for c in range(2):
                    nc.tensor.matmul(
                        et[:, c * 512 : (c + 1) * 512],
                        U[:, c * 128 : (c + 1) * 128],
                        ohb,
                        start=False,
                        stop=True,
                        skip_group_check=True,
                    )

                # evacuate relu(z) -> SBUF  (split 3:2 across ACT / DVE)
                ht = hpool.tile([128, 1024], BF16, tag="h")
                if b % 5 < 3:
                    nc.scalar.activation(ht[:], et[:], AF.Relu)
                else:
                    nc.vector.tensor_scalar_max(ht[:], et[:], 0.0)
                hts[b % 4] = ht

                # stage 2: out2[32j+2s+o, a] += sum_h W2[h,o] * h[h,a]
                # issued in groups of 4 batches so the 4 column-strip matmuls
                # are back-to-back and run concurrently on the PE sub-arrays
                gg = b % block
                if gg == 0:
                    o2 = o2pool.tile([128, 512], F32, tag="o2")
                if gg % 4 == 3:
                    s = gg // 4
                    for c in range(2):
                        for j in range(4):
                            nc.tensor.matmul(
                                o2[32 * j : 32 * j + 32, :],
                                w2sl[c][:, 32 * s : 32 * s + 32],
                                hts[j][:, c * 512 : (c + 1) * 512],
                                start=(s == 0 and c == 0),
                                stop=(s == block // 4 - 1 and c == 1),
                                skip_group_check=True,
                                tile_position=(0, 32 * j),
                            )

                if gg == block - 1:
                    blk = b // block
                    outsb = opool.tile([128, 512], F32, tag="outsb")
                    # + b2 (per-partition scalar: b2[o] at partition 32j+2s+o)
                    nc.vector.tensor_scalar(
                        outsb[:], o2[:], b2c[:], None, op0=ALU.add
                    )
                    nc.sync.dma_start(out_d[blk], outsb[:])

    nc.finalize()
    return nc


_CACHE = {}


def _get_module(nb: int):
    if nb not in _CACHE:
        _CACHE[nb] = _build(nb)
    return _CACHE[nb]


def _build_noop(nb: int):
    """Same I/O signature as _build but only copies one tile — used to
    measure the fixed dispatch/transfer overhead of a call."""
    block = min(64, nb)
    nc = bacc.Bacc(
        "TRN2", target_bir_lowering=False, debug=False, num_devices=NCORES
    )
    nc.dram_tensor("ohpm", [nb, 64, 512], BF16, kind="ExternalInput")
    nc.dram_tensor("u64", [nb, 64, H], BF16, kind="ExternalInput")
    nc.dram_tensor("w1h", [E, H], F32, kind="ExternalInput")
    nc.dram_tensor("b1r", [1, H], F32, kind="ExternalInput")
    nc.dram_tensor("embT", [E, A], F32, kind="ExternalInput")
    w2sl_d = nc.dram_tensor("w2sl", [2, 128, 512], BF16, kind="ExternalInput").ap()
    nc.dram_tensor("b2c", [128, 1], F32, kind="ExternalInput")
    out_d = nc.dram_tensor(
        "out", [nb // block, 128, 512], F32, kind="ExternalOutput"
    ).ap()
    with tile.TileContext(nc) as tc:
        with tc.tile_pool(name="sb", bufs=1) as pool:
            t = pool.tile([128, 512], BF16, tag="t")
            nc.sync.dma_start(t[:], w2sl_d[0])
            for blk in range(nb // block):
                nc.sync.dma_start(out_d[blk], t[:])
    nc.finalize()
    return nc


def noop_time(inputs, _nb: int = BC):
    nb = _nb
    key = ("noop", nb)
    if key not in _CACHE:
        _CACHE[key] = _build_noop(nb)
    nc = _CACHE[key]
    in_maps = _prep_host(
        inputs["state"], inputs["abs_actions"],
        inputs["abstract_agent_assignments"], inputs["embed_table"],
        inputs["W1"], inputs["b1"], inputs["W2"], inputs["b2"], nb,
    )
    bass_utils.run_bass_kernel_spmd(nc, in_maps, core_ids=list(range(NCORES)))


def _prep_host(state, abs_actions, assignments, embed_table, W1, b1, W2, b2, nb):
    """Build the per-core input maps (host-side data marshaling only)."""
    idx = np.asarray(assignments).astype(np.int32)  # values < 16
    absf = np.asarray(abs_actions, dtype=np.float32)
    W1 = np.asarray(W1, dtype=np.float32)
    W2 = np.asarray(W2, dtype=np.float32)
    b1 = np.asarray(b1, dtype=np.float32)
    b2 = np.asarray(b2, dtype=np.float32)
    emb = np.asarray(embed_table, dtype=np.float32)

    # constants shared by all cores
    w1h = W1[1:, :].copy()  # [256, 256]
    b1r = b1.reshape(1, H)
    embT = emb.T.copy()  # [256, 512]
    w2sl = np.zeros((2, 128, 512), np.float32)
    for c in range(2):
        for s in range(16):
            for o in range(OUT):
                w2sl[c, :, 32 * s + 2 * s + o] = W2[128 * c : 128 * (c + 1), o]
    w2sl = w2sl.astype(ml_dtypes.bfloat16)
    b2c = np.zeros((128, 1), np.float32)
    for j in range(4):
        for s in range(16):
            for o in range(OUT):
                b2c[32 * j + 2 * s + o, 0] = b2[o]

    # one-hot of the assignments, [B, 16, 512] f32
    oh = (idx[:, None, :] == np.arange(NABS, dtype=np.int32)[None, :, None]).astype(
        np.float32
    )

    in_maps = []
    for m in range(NCORES):
        rows = slice(m * BC, m * BC + nb)
        ohc = oh[rows]  # [nb, 16, 512]
        ohpm = np.zeros((nb, 64, 512), ml_dtypes.bfloat16)
        ohpm[:, 16:32, :] = ohc
        ohpm[NE:, 0:16, :] = -ohc[:-NE]
        ohpm[:, 32:64, :] = ohpm[:, 0:32, :]
        # U stacked [prev; cur] then split hi/lo so the rank-64 matmul is
        # exact: U64 = [hi(32); lo(32)], p = outer(abs, w1row) in f32
        absc = absf[rows]  # [nb, 16]
        ab32 = np.zeros((nb, 32), np.float32)
        ab32[:, 16:32] = absc
        ab32[NE:, 0:16] = absc[:-NE]
        p = ab32[:, :, None] * W1[0][None, None, :]  # [nb, 32, H] f32
        hi = p.astype(ml_dtypes.bfloat16)
        lo = (p - hi.astype(np.float32)).astype(ml_dtypes.bfloat16)
        u64 = np.concatenate([hi, lo], axis=1)  # [nb, 64, H]
        in_maps.append(
            {
                "ohpm": ohpm,
                "u64": u64,
                "w1h": w1h,
                "b1r": b1r,
                "embT": embT,
                "w2sl": w2sl,
                "b2c": b2c,
            }
        )
    return in_maps


def kernel(
    state,
    abs_actions,
    abstract_agent_assignments,
    embed_table,
    W1,
    b1,
    W2,
    b2,
    _nb: int = BC,
):
    nb = _nb
    nc = _get_module(nb)
    in_maps = _prep_host(
        state, abs_actions, abstract_agent_assignments,
        embed_table, W1, b1, W2, b2, nb,
    )
    res = bass_utils.run_bass_kernel_spmd(nc, in_maps, core_ids=list(range(NCORES)))
    block = min(64, nb)
    full = np.zeros((B, A, OUT), np.float32)
    for m in range(NCORES):
        scr = res.results[m]["out"]  # [nb//block, 128, 512]
        v = scr.reshape(nb // block, 4, 16, OUT, A)  # p = 32j + 2s + o
        v = v.transpose(0, 2, 1, 4, 3)  # [blk, s, j, a, o]
        full[m * BC : m * BC + nb] = v.reshape(-1, A, OUT)[:nb]
    return full


# revision 16
# speedup vs baseline: 1.0132x; 1.0132x over previous
"""Trainium2 Bass kernel for nn_Decoder (gnn_message_passing).

Math (per batch b, agent a):
    s[b,a]  = abs_actions[b, idx[b,a]]                     (gather, idx < 16)
    z[b,a,:] = s[b,a] * W1[0,:] + embed[a,:] @ W1[1:,:] + b1
    out[b,a,:] = relu(z) @ W2 + b2

Device algorithm (per core, hT layout z[h, a], pure data-parallel over B):
  - e[h,a] = (embed @ W1[1:]).T + b1 is batch-independent; it is computed
    once on device (matmuls from W1h / embT) into 3 PSUM tiles that stay
    RESIDENT for the whole kernel.
  - The gather is folded into a rank-64 matmul: the per-batch term is
    v_b = U_b.T @ onehot_b with U_b = outer(abs_row_b, W1[0]); the host ships
    the one-hot encodings (bf16 exact) and U split into bf16 hi/lo halves so
    the bf16 matmul reproduces the f32 product almost exactly.  Each batch
    issues a single "transition" matmul pair (2 h-chunks, K=64) whose
    stacked rhs holds [-onehot_{b-3}; +onehot_b] twice: it simultaneously
    removes the previous occupant's contribution from the rotating PSUM tile
    and adds the new batch's, so PSUM always holds z = e + v_b right after.
    fp32 PSUM makes the add/remove round-trip drift negligible (~1e-7).
  - relu evacuation PSUM->SBUF alternates between ScalarE (ACTIVATE Relu)
    and VectorE (tensor_scalar max 0), the two engines that can read PSUM.
  - Stage 2 (h @ W2, OUT=2) packs 64 batches into one PSUM bank: W2 sits in
    zero-padded 32-column "slot" tiles targeting column-strip j via
    tile_position=(0,32j); batch gg lands on partitions 32j+2s+{0,1}.
    One copy + DMA evacuates 64 batches of output at once; the host
    unpermutes the [blocks,128,512] scratch layout.
"""

import numpy as np
import ml_dtypes

import concourse.bass as bass
import concourse.bacc as bacc
import concourse.mybir as mybir
import concourse.tile as tile
from concourse import bass_utils

F32 = mybir.dt.float32
BF16 = mybir.dt.bfloat16

B, A, NABS, E, H, OUT = 2048, 512, 16, 256, 256, 2
NCORES = 8
BC = B // NCORES  # batches per core
NE = 3  # rotating resident-e PSUM tiles

AF = mybir.ActivationFunctionType
ALU = mybir.AluOpType


def _build(nb: int):
    """Build the per-core module processing nb batches."""
    assert nb % 4 == 0
    block = min(64, nb)  # batches accumulated per stage-2 psum bank
    nc = bacc.Bacc(
        "TRN2", target_bir_lowering=False, debug=False, num_devices=NCORES
    )

    ohpm_d = nc.dram_tensor("ohpm", [nb, 64, 512], BF16, kind="ExternalInput").ap()
    u64_d = nc.dram_tensor("u64", [nb, 64, H], BF16, kind="ExternalInput").ap()
    w1h_d = nc.dram_tensor("w1h", [E, H], F32, kind="ExternalInput").ap()
    b1_d = nc.dram_tensor("b1r", [1, H], F32, kind="ExternalInput").ap()
    embT_d = nc.dram_tensor("embT", [E, A], F32, kind="ExternalInput").ap()
    w2sl_d = nc.dram_tensor("w2sl", [2, 128, 512], BF16, kind="ExternalInput").ap()
    b2c_d = nc.dram_tensor("b2c", [128, 1], F32, kind="ExternalInput").ap()
    out_d = nc.dram_tensor(
        "out", [nb // block, 128, 512], F32, kind="ExternalOutput"
    ).ap()

    with tile.TileContext(nc) as tc:
        with (
            tc.tile_pool(name="const", bufs=1) as cpool,
            tc.tile_pool(name="ohb", bufs=6) as ohpool,
            tc.tile_pool(name="u", bufs=4) as upool,
            tc.tile_pool(name="h", bufs=6) as hpool,
            tc.tile_pool(name="osb", bufs=2) as opool,
            tc.tile_pool(name="epool", bufs=NE, space="PSUM") as epool,
            tc.tile_pool(name="o2", bufs=2, space="PSUM") as o2pool,
        ):
            # ---- resident constants ----
            w1h0 = cpool.tile([128, H], F32, tag="w1h0")
            nc.sync.dma_start(w1h0[:], w1h_d[0:128, :])
            w1h1 = cpool.tile([128, H], F32, tag="w1h1")
            nc.sync.dma_start(w1h1[:], w1h_d[128:256, :])
            b1sb = cpool.tile([1, H], F32, tag="b1sb")
            nc.sync.dma_start(b1sb[:], b1_d[:])
            embT0 = cpool.tile([128, A], F32, tag="embT0")
            nc.sync.dma_start(embT0[:], embT_d[0:128, :])
            embT1 = cpool.tile([128, A], F32, tag="embT1")
            nc.sync.dma_start(embT1[:], embT_d[128:256, :])
            w2sl0 = cpool.tile([128, 512], BF16, tag="w2sl0")
            nc.sync.dma_start(w2sl0[:], w2sl_d[0])
            w2sl1 = cpool.tile([128, 512], BF16, tag="w2sl1")
            nc.sync.dma_start(w2sl1[:], w2sl_d[1])
            b2c = cpool.tile([128, 1], F32, tag="b2c")
            nc.sync.dma_start(b2c[:], b2c_d[:])
            ones = cpool.tile([1, A], F32, tag="ones")
            nc.vector.memset(ones[:], 1.0)

            w1h = [w1h0, w1h1]
            embT = [embT0, embT1]
            w2sl = [w2sl0, w2sl1]

            # ---- seed the resident e tiles:  e[h, a] = W1h.T @ embT + b1 ----
            etiles = [
                epool.tile([128, 1024], F32, tag="e", name=f"etile{t}")
                for t in range(NE)
            ]
            for et in etiles:
                for c in range(2):  # h-chunk
                    for c2 in range(2):  # contraction (embedding dim) chunk
                        nc.tensor.matmul(
                            et[:, c * 512 : (c + 1) * 512],
                            w1h[c2][:, c * 128 : (c + 1) * 128],
                            embT[c2][:],
                            start=(c2 == 0),
                            stop=False,
                            skip_group_check=True,
                        )
                    nc.tensor.matmul(
                        et[:, c * 512 : (c + 1) * 512],
                        b1sb[:, c * 128 : (c + 1) * 128],
                        ones[:],
                        start=False,
                        stop=True,
                        skip_group_check=True,
                    )

            # ---- batch loop ----
            o2 = None
            ohb8 = None
            u8 = None
            hts = [None] * 4
            for b in range(nb):
                if b % 8 == 0:
                    ng = min(8, nb - b)
                    ohb8 = ohpool.tile([64, 4096], BF16, tag="ohb")
                    nc.sync.dma_start(
                        ohb8[:, 0 : ng * 512].rearrange("p (t c) -> p t c", t=ng),
                        ohpm_d[b : b + ng].rearrange("t p c -> p t c"),
                    )
                    u8 = upool.tile([64, 8 * H], BF16, tag="U")
                    nc.sync.dma_start(
                        u8[:, 0 : ng * H].rearrange("p (t c) -> p t c", t=ng),
                        u64_d[b : b + ng].rearrange("t p c -> p t c"),
                    )
                ohb = ohb8[:, (b % 8) * 512 : (b % 8) * 512 + 512]
                U = u8[:, (b % 8) * H : (b % 8) * H + H]

                et = etiles[b % NE]
                # transition: E += -U_{b-NE}.T @ oh_{b-NE} + U_b.T @ oh_b
                for c in range(2):
                    nc.tensor.matmul(
                        et[:, c * 512 : (c + 1) * 512],
                        U[:, c * 128 : (c + 1) * 128],
                        ohb,
                        start=False,
                        stop=True,
                        skip_group_check=True,
                    )

                # evacuate relu(z) -> SBUF  (split 3:2 across ACT / DVE)
                ht = hpool.tile([128, 1024], BF16, tag="h")
                if (b % 9) % 2 == 0:
                    nc.scalar.activation(ht[:], et[:], AF.Relu)
                else:
                    nc.vector.tensor_scalar_max(ht[:], et[:], 0.0)
                hts[b % 4] = ht

                # stage 2: out2[32j+2s+o, a] += sum_h W2[h,o] * h[h,a]
                # issued in groups of 4 batches so the 4 column-strip matmuls
                # are back-to-back and run concurrently on the PE sub-arrays
                gg = b % block
                if gg == 0:
                    o2 = o2pool.tile([128, 512], F32, tag="o2")
                if gg % 4 == 3:
                    s = gg // 4
                    for c in range(2):
                        for j in range(4):
                            nc.tensor.matmul(
                                o2[32 * j : 32 * j + 32, :],
                                w2sl[c][:, 32 * s : 32 * s + 32],
                                hts[j][:, c * 512 : (c + 1) * 512],
                                start=(s == 0 and c == 0),
                                stop=(s == block // 4 - 1 and c == 1),
                                skip_group_check=True,
                                tile_position=(0, 32 * j),
                            )

                if gg == block - 1:
                    blk = b // block
                    outsb = opool.tile([128, 512], F32, tag="outsb")
                    # + b2 (per-partition scalar: b2[o] at partition 32j+2s+o)
                    nc.vector.tensor_scalar(
                        outsb[:], o2[:], b2c[:], None, op0=ALU.add
                    )
                    nc.sync.dma_start(out_d[blk], outsb[:])

    nc.finalize()
    return nc


_CACHE = {}


def _get_module(nb: int):
    if nb not in _CACHE:
        _CACHE[nb] = _build(nb)
    return _CACHE[nb]


def _build_noop(nb: int):
    """Same I/O signature as _build but only copies one tile — used to
    measure the fixed dispatch/transfer overhead of a call."""
    block = min(64, nb)
    nc = bacc.Bacc(
        "TRN2", target_bir_lowering=False, debug=False, num_devices=NCORES
    )
    nc.dram_tensor("ohpm", [nb, 64, 512], BF16, kind="ExternalInput")
    nc.dram_tensor("u64", [nb, 64, H], BF16, kind="ExternalInput")
    nc.dram_tensor("w1h", [E, H], F32, kind="ExternalInput")
    nc.dram_tensor("b1r", [1, H], F32, kind="ExternalInput")
    nc.dram_tensor("embT", [E, A], F32, kind="ExternalInput")
    w2sl_d = nc.dram_tensor("w2sl", [2, 128, 512], BF16, kind="ExternalInput").ap()
    nc.dram_tensor("b2c", [128, 1], F32, kind="ExternalInput")
    out_d = nc.dram_tensor(
        "out", [nb // block, 128, 512], F32, kind="ExternalOutput"
    ).ap()
    with tile.TileContext(nc) as tc:
        with tc.tile_pool(name="sb", bufs=1) as pool:
            t = pool.tile([128, 512], BF16, tag="t")
            nc.sync.dma_start(t[:], w2sl_d[0])
            for blk in range(nb // block):
                nc.sync.dma_start(out_d[blk], t[:])
    nc.finalize()
    return nc


def noop_time(inputs, _nb: int = BC):
    nb = _nb
    key = ("noop", nb)
    if key not in _CACHE:
        _CACHE[key] = _build_noop(nb)
    nc = _CACHE[key]
    in_maps = _prep_host(
        inputs["state"], inputs["abs_actions"],
        inputs["abstract_agent_assignments"], inputs["embed_table"],
        inputs["W1"], inputs["b1"], inputs["W2"], inputs["b2"], nb,
    )
    bass_utils.run_bass_kernel_spmd(nc, in_maps, core_ids=list(range(NCORES)))


def _prep_host(state, abs_actions, assignments, embed_table, W1, b1, W2, b2, nb):
    """Build the per-core input maps (host-side data marshaling only)."""
    idx = np.asarray(assignments).astype(np.int32)  # values < 16
    absf = np.asarray(abs_actions, dtype=np.float32)
    W1 = np.asarray(W1, dtype=np.float32)
    W2 = np.asarray(W2, dtype=np.float32)
    b1 = np.asarray(b1, dtype=np.float32)
    b2 = np.asarray(b2, dtype=np.float32)
    emb = np.asarray(embed_table, dtype=np.float32)

    # constants shared by all cores
    w1h = W1[1:, :].copy()  # [256, 256]
    b1r = b1.reshape(1, H)
    embT = emb.T.copy()  # [256, 512]
    w2sl = np.zeros((2, 128, 512), np.float32)
    for c in range(2):
        for s in range(16):
            for o in range(OUT):
                w2sl[c, :, 32 * s + 2 * s + o] = W2[128 * c : 128 * (c + 1), o]
    w2sl = w2sl.astype(ml_dtypes.bfloat16)
    b2c = np.zeros((128, 1), np.float32)
    for j in range(4):
        for s in range(16):
            for o in range(OUT):
                b2c[32 * j + 2 * s + o, 0] = b2[o]

    # one-hot of the assignments, [B, 16, 512] f32
    oh = (idx[:, None, :] == np.arange(NABS, dtype=np.int32)[None, :, None]).astype(
        np.float32
    )

    in_maps = []
    for m in range(NCORES):
        rows = slice(m * BC, m * BC + nb)
        ohc = oh[rows]  # [nb, 16, 512]
        ohpm = np.zeros((nb, 64, 512), ml_dtypes.bfloat16)
        ohpm[:, 16:32, :] = ohc
        ohpm[NE:, 0:16, :] = -ohc[:-NE]
        ohpm[:, 32:64, :] = ohpm[:, 0:32, :]
        # U stacked [prev; cur] then split hi/lo so the rank-64 matmul is
        # exact: U64 = [hi(32); lo(32)], p = outer(abs, w1row) in f32
        absc = absf[rows]  # [nb, 16]
        ab32 = np.zeros((nb, 32), np.float32)
        ab32[:, 16:32] = absc
        ab32[NE:, 0:16] = absc[:-NE]
        p = ab32[:, :, None] * W1[0][None, None, :]  # [nb, 32, H] f32
        hi = p.astype(ml_dtypes.bfloat16)
        lo = (p - hi.astype(np.float32)).astype(ml_dtypes.bfloat16)
        u64 = np.concatenate([hi, lo], axis=1)  # [nb, 64, H]
        in_maps.append(
            {
                "ohpm": ohpm,
                "u64": u64,
                "w1h": w1h,
                "b1r": b1r,
                "embT": embT,
                "w2sl": w2sl,
                "b2c": b2c,
            }
        )
    return in_maps


def kernel(
    state,
    abs_actions,
    abstract_agent_assignments,
    embed_table,
    W1,
    b1,
    W2,
    b2,
    _nb: int = BC,
):
    nb = _nb
    nc = _get_module(nb)
    in_maps = _prep_host(
        state, abs_actions, abstract_agent_assignments,
        embed_table, W1, b1, W2, b2, nb,
    )
    res = bass_utils.run_bass_kernel_spmd(nc, in_maps, core_ids=list(range(NCORES)))
    block = min(64, nb)
    full = np.zeros((B, A, OUT), np.float32)
    for m in range(NCORES):
        scr = res.results[m]["out"]  # [nb//block, 128, 512]
        v = scr.reshape(nb // block, 4, 16, OUT, A)  # p = 32j + 2s + o
        v = v.transpose(0, 2, 1, 4, 3)  # [blk, s, j, a, o]
        full[m * BC : m * BC + nb] = v.reshape(-1, A, OUT)[:nb]
    return full


# revision 17
# speedup vs baseline: 1.0923x; 1.0780x over previous
"""Trainium2 Bass kernel for nn_Decoder (gnn_message_passing).

Math (per batch b, agent a):
    s[b,a]  = abs_actions[b, idx[b,a]]                     (gather, idx < 16)
    z[b,a,:] = s[b,a] * W1[0,:] + embed[a,:] @ W1[1:,:] + b1
    out[b,a,:] = relu(z) @ W2 + b2

Device algorithm (per core, hT layout z[h, a], pure data-parallel over B):
  - e[h,a] = (embed @ W1[1:]).T + b1 is batch-independent; it is computed
    once on device (matmuls from W1h / embT) into 3 PSUM tiles that stay
    RESIDENT for the whole kernel.
  - The gather is folded into a rank-64 matmul: the per-batch term is
    v_b = U_b.T @ onehot_b with U_b = outer(abs_row_b, W1[0]); the host ships
    the one-hot encodings (bf16 exact) and U split into bf16 hi/lo halves so
    the bf16 matmul reproduces the f32 product almost exactly.  Each batch
    issues a single "transition" matmul pair (2 h-chunks, K=64) whose
    stacked rhs holds [-onehot_{b-3}; +onehot_b] twice: it simultaneously
    removes the previous occupant's contribution from the rotating PSUM tile
    and adds the new batch's, so PSUM always holds z = e + v_b right after.
    fp32 PSUM makes the add/remove round-trip drift negligible (~1e-7).
  - relu evacuation PSUM->SBUF alternates between ScalarE (ACTIVATE Relu)
    and VectorE (tensor_scalar max 0), the two engines that can read PSUM.
  - Stage 2 (h @ W2, OUT=2) packs 64 batches into one PSUM bank: W2 sits in
    zero-padded 32-column "slot" tiles targeting column-strip j via
    tile_position=(0,32j); batch gg lands on partitions 32j+2s+{0,1}.
    One copy + DMA evacuates 64 batches of output at once; the host
    unpermutes the [blocks,128,512] scratch layout.
"""

import numpy as np
import ml_dtypes

import concourse.bass as bass
import concourse.bacc as bacc
import concourse.mybir as mybir
import concourse.tile as tile
from concourse import bass_utils

F32 = mybir.dt.float32
BF16 = mybir.dt.bfloat16

B, A, NABS, E, H, OUT = 2048, 512, 16, 256, 256, 2
NCORES = 8
BC = B // NCORES  # batches per core
NE = 3  # rotating resident-e PSUM tiles

AF = mybir.ActivationFunctionType
ALU = mybir.AluOpType


def _build(nb: int):
    """Build the per-core module processing nb batches."""
    assert nb % 4 == 0
    block = min(64, nb)  # batches accumulated per stage-2 psum bank
    nc = bacc.Bacc(
        "TRN2", target_bir_lowering=False, debug=False, num_devices=NCORES
    )

    ohpm_d = nc.dram_tensor("ohpm", [nb, 64, 512], BF16, kind="ExternalInput").ap()
    u64_d = nc.dram_tensor("u64", [nb, 64, H], BF16, kind="ExternalInput").ap()
    w1hx_d = nc.dram_tensor("w1hx", [2, E, H], BF16, kind="ExternalInput").ap()
    b1x_d = nc.dram_tensor("b1x", [2, 1, H], BF16, kind="ExternalInput").ap()
    embTx_d = nc.dram_tensor("embTx", [2, E, A], BF16, kind="ExternalInput").ap()
    w2sl_d = nc.dram_tensor("w2sl", [2, 128, 512], BF16, kind="ExternalInput").ap()
    b2c_d = nc.dram_tensor("b2c", [128, 1], F32, kind="ExternalInput").ap()
    out_d = nc.dram_tensor(
        "out", [nb // block, 128, 512], F32, kind="ExternalOutput"
    ).ap()

    with tile.TileContext(nc) as tc:
        with (
            tc.tile_pool(name="const", bufs=1) as cpool,
            tc.tile_pool(name="ohb", bufs=6) as ohpool,
            tc.tile_pool(name="u", bufs=4) as upool,
            tc.tile_pool(name="h", bufs=6) as hpool,
            tc.tile_pool(name="osb", bufs=2) as opool,
            tc.tile_pool(name="epool", bufs=NE, space="PSUM") as epool,
            tc.tile_pool(name="o2", bufs=2, space="PSUM") as o2pool,
        ):
            # ---- resident constants (hi/lo bf16 halves of the f32 data) ----
            w1hs = []
            embTs = []
            b1s = []
            for hl in range(2):
                for c2 in range(2):
                    t = cpool.tile([128, H], BF16, name=f"w1h_{hl}_{c2}",
                                   tag=f"w1h{hl}{c2}")
                    nc.sync.dma_start(t[:], w1hx_d[hl, c2 * 128 : (c2 + 1) * 128, :])
                    w1hs.append(t)
                    t = cpool.tile([128, A], BF16, name=f"embT_{hl}_{c2}",
                                   tag=f"embT{hl}{c2}")
                    nc.sync.dma_start(t[:], embTx_d[hl, c2 * 128 : (c2 + 1) * 128, :])
                    embTs.append(t)
                t = cpool.tile([1, H], BF16, name=f"b1_{hl}", tag=f"b1{hl}")
                nc.sync.dma_start(t[:], b1x_d[hl])
                b1s.append(t)
            w2sl0 = cpool.tile([128, 512], BF16, tag="w2sl0")
            nc.sync.dma_start(w2sl0[:], w2sl_d[0])
            w2sl1 = cpool.tile([128, 512], BF16, tag="w2sl1")
            nc.sync.dma_start(w2sl1[:], w2sl_d[1])
            b2c = cpool.tile([128, 1], F32, tag="b2c")
            nc.sync.dma_start(b2c[:], b2c_d[:])
            ones = cpool.tile([1, A], BF16, tag="ones")
            nc.vector.memset(ones[:], 1.0)

            w2sl = [w2sl0, w2sl1]

            # ---- seed the resident e tiles:  e[h, a] = W1h.T @ embT + b1 ----
            etiles = [
                epool.tile([128, 1024], F32, tag="e", name=f"etile{t}")
                for t in range(NE)
            ]
            for et in etiles:
                for c in range(2):  # h-chunk
                    first = True
                    for c2 in range(2):  # contraction (embedding dim) chunk
                        # (Whi+Wlo)@(Ehi+Elo) ~= hi@hi + hi@lo + lo@hi
                        for wl, el in ((0, 0), (0, 1), (1, 0)):
                            nc.tensor.matmul(
                                et[:, c * 512 : (c + 1) * 512],
                                w1hs[2 * wl + c2][:, c * 128 : (c + 1) * 128],
                                embTs[2 * el + c2][:],
                                start=first,
                                stop=False,
                                skip_group_check=True,
                            )
                            first = False
                    for hl in range(2):
                        nc.tensor.matmul(
                            et[:, c * 512 : (c + 1) * 512],
                            b1s[hl][:, c * 128 : (c + 1) * 128],
                            ones[:],
                            start=False,
                            stop=(hl == 1),
                            skip_group_check=True,
                        )

            # ---- batch loop ----
            o2 = None
            ohb8 = None
            u8 = None
            hts = [None] * 4
            for b in range(nb):
                if b % 8 == 0:
                    ng = min(8, nb - b)
                    ohb8 = ohpool.tile([64, 4096], BF16, tag="ohb")
                    nc.sync.dma_start(
                        ohb8[:, 0 : ng * 512].rearrange("p (t c) -> p t c", t=ng),
                        ohpm_d[b : b + ng].rearrange("t p c -> p t c"),
                    )
                    u8 = upool.tile([64, 8 * H], BF16, tag="U")
                    nc.sync.dma_start(
                        u8[:, 0 : ng * H].rearrange("p (t c) -> p t c", t=ng),
                        u64_d[b : b + ng].rearrange("t p c -> p t c"),
                    )
                ohb = ohb8[:, (b % 8) * 512 : (b % 8) * 512 + 512]
                U = u8[:, (b % 8) * H : (b % 8) * H + H]

                et = etiles[b % NE]
                # transition: E += -U_{b-NE}.T @ oh_{b-NE} + U_b.T @ oh_b
                for c in range(2):
                    nc.tensor.matmul(
                        et[:, c * 512 : (c + 1) * 512],
                        U[:, c * 128 : (c + 1) * 128],
                        ohb,
                        start=False,
                        stop=True,
                        skip_group_check=True,
                    )

                # evacuate relu(z) -> SBUF  (split 3:2 across ACT / DVE)
                ht = hpool.tile([128, 1024], BF16, tag="h")
                if (b % 9) % 2 == 0:
                    nc.scalar.activation(ht[:], et[:], AF.Relu)
                else:
                    nc.vector.tensor_scalar_max(ht[:], et[:], 0.0)
                hts[b % 4] = ht

                # stage 2: out2[32j+2s+o, a] += sum_h W2[h,o] * h[h,a]
                # issued in groups of 4 batches so the 4 column-strip matmuls
                # are back-to-back and run concurrently on the PE sub-arrays
                gg = b % block
                if gg == 0:
                    o2 = o2pool.tile([128, 512], F32, tag="o2")
                if gg % 4 == 3:
                    s = gg // 4
                    for c in range(2):
                        for j in range(4):
                            nc.tensor.matmul(
                                o2[32 * j : 32 * j + 32, :],
                                w2sl[c][:, 32 * s : 32 * s + 32],
                                hts[j][:, c * 512 : (c + 1) * 512],
                                start=(s == 0 and c == 0),
                                stop=(s == block // 4 - 1 and c == 1),
                                skip_group_check=True,
                                tile_position=(0, 32 * j),
                            )

                if gg == block - 1:
                    blk = b // block
                    outsb = opool.tile([128, 512], F32, tag="outsb")
                    # + b2 (per-partition scalar: b2[o] at partition 32j+2s+o)
                    nc.vector.tensor_scalar(
                        outsb[:], o2[:], b2c[:], None, op0=ALU.add
                    )
                    nc.sync.dma_start(out_d[blk], outsb[:])

    nc.finalize()
    return nc


_CACHE = {}


def _get_module(nb: int):
    if nb not in _CACHE:
        _CACHE[nb] = _build(nb)
    return _CACHE[nb]


def _build_noop(nb: int):
    """Same I/O signature as _build but only copies one tile — used to
    measure the fixed dispatch/transfer overhead of a call."""
    block = min(64, nb)
    nc = bacc.Bacc(
        "TRN2", target_bir_lowering=False, debug=False, num_devices=NCORES
    )
    nc.dram_tensor("ohpm", [nb, 64, 512], BF16, kind="ExternalInput")
    nc.dram_tensor("u64", [nb, 64, H], BF16, kind="ExternalInput")
    nc.dram_tensor("w1hx", [2, E, H], BF16, kind="ExternalInput")
    nc.dram_tensor("b1x", [2, 1, H], BF16, kind="ExternalInput")
    nc.dram_tensor("embTx", [2, E, A], BF16, kind="ExternalInput")
    w2sl_d = nc.dram_tensor("w2sl", [2, 128, 512], BF16, kind="ExternalInput").ap()
    nc.dram_tensor("b2c", [128, 1], F32, kind="ExternalInput")
    out_d = nc.dram_tensor(
        "out", [nb // block, 128, 512], F32, kind="ExternalOutput"
    ).ap()
    with tile.TileContext(nc) as tc:
        with tc.tile_pool(name="sb", bufs=1) as pool:
            t = pool.tile([128, 512], BF16, tag="t")
            nc.sync.dma_start(t[:], w2sl_d[0])
            for blk in range(nb // block):
                nc.sync.dma_start(out_d[blk], t[:])
    nc.finalize()
    return nc


def noop_time(inputs, _nb: int = BC):
    nb = _nb
    key = ("noop", nb)
    if key not in _CACHE:
        _CACHE[key] = _build_noop(nb)
    nc = _CACHE[key]
    in_maps = _prep_host(
        inputs["state"], inputs["abs_actions"],
        inputs["abstract_agent_assignments"], inputs["embed_table"],
        inputs["W1"], inputs["b1"], inputs["W2"], inputs["b2"], nb,
    )
    bass_utils.run_bass_kernel_spmd(nc, in_maps, core_ids=list(range(NCORES)))


def _prep_host(state, abs_actions, assignments, embed_table, W1, b1, W2, b2, nb):
    """Build the per-core input maps (host-side data marshaling only)."""
    idx = np.asarray(assignments).astype(np.int32)  # values < 16
    absf = np.asarray(abs_actions, dtype=np.float32)
    W1 = np.asarray(W1, dtype=np.float32)
    W2 = np.asarray(W2, dtype=np.float32)
    b1 = np.asarray(b1, dtype=np.float32)
    b2 = np.asarray(b2, dtype=np.float32)
    emb = np.asarray(embed_table, dtype=np.float32)

    # constants shared by all cores, split into bf16 hi/lo halves
    def hilo(x):
        hi = x.astype(ml_dtypes.bfloat16)
        lo = (x - hi.astype(np.float32)).astype(ml_dtypes.bfloat16)
        return np.stack([hi, lo])

    w1hx = hilo(W1[1:, :])  # [2, 256, 256]
    b1x = hilo(b1.reshape(1, H))  # [2, 1, 256]
    embTx = hilo(emb.T.copy())  # [2, 256, 512]
    w2sl = np.zeros((2, 128, 512), np.float32)
    for c in range(2):
        for s in range(16):
            for o in range(OUT):
                w2sl[c, :, 32 * s + 2 * s + o] = W2[128 * c : 128 * (c + 1), o]
    w2sl = w2sl.astype(ml_dtypes.bfloat16)
    b2c = np.zeros((128, 1), np.float32)
    for j in range(4):
        for s in range(16):
            for o in range(OUT):
                b2c[32 * j + 2 * s + o, 0] = b2[o]

    # one-hot of the assignments, [B, 16, 512] f32
    oh = (idx[:, None, :] == np.arange(NABS, dtype=np.int32)[None, :, None]).astype(
        np.float32
    )

    in_maps = []
    for m in range(NCORES):
        rows = slice(m * BC, m * BC + nb)
        ohc = oh[rows]  # [nb, 16, 512]
        ohpm = np.zeros((nb, 64, 512), ml_dtypes.bfloat16)
        ohpm[:, 16:32, :] = ohc
        ohpm[NE:, 0:16, :] = -ohc[:-NE]
        ohpm[:, 32:64, :] = ohpm[:, 0:32, :]
        # U stacked [prev; cur] then split hi/lo so the rank-64 matmul is
        # exact: U64 = [hi(32); lo(32)], p = outer(abs, w1row) in f32
        absc = absf[rows]  # [nb, 16]
        ab32 = np.zeros((nb, 32), np.float32)
        ab32[:, 16:32] = absc
        ab32[NE:, 0:16] = absc[:-NE]
        p = ab32[:, :, None] * W1[0][None, None, :]  # [nb, 32, H] f32
        hi = p.astype(ml_dtypes.bfloat16)
        lo = (p - hi.astype(np.float32)).astype(ml_dtypes.bfloat16)
        u64 = np.concatenate([hi, lo], axis=1)  # [nb, 64, H]
        in_maps.append(
            {
                "ohpm": ohpm,
                "u64": u64,
                "w1hx": w1hx,
                "b1x": b1x,
                "embTx": embTx,
                "w2sl": w2sl,
                "b2c": b2c,
            }
        )
    return in_maps


def kernel(
    state,
    abs_actions,
    abstract_agent_assignments,
    embed_table,
    W1,
    b1,
    W2,
    b2,
    _nb: int = BC,
):
    nb = _nb
    nc = _get_module(nb)
    in_maps = _prep_host(
        state, abs_actions, abstract_agent_assignments,
        embed_table, W1, b1, W2, b2, nb,
    )
    res = bass_utils.run_bass_kernel_spmd(nc, in_maps, core_ids=list(range(NCORES)))
    block = min(64, nb)
    full = np.zeros((B, A, OUT), np.float32)
    for m in range(NCORES):
        scr = res.results[m]["out"]  # [nb//block, 128, 512]
        v = scr.reshape(nb // block, 4, 16, OUT, A)  # p = 32j + 2s + o
        v = v.transpose(0, 2, 1, 4, 3)  # [blk, s, j, a, o]
        full[m * BC : m * BC + nb] = v.reshape(-1, A, OUT)[:nb]
    return full
